# revision 12
# baseline (speedup 1.0000x reference)
"""Grouped-Query Attention Trainium2 kernel (8 NeuronCores, SPMD).

Problem: B=2, S=2048, D=1024, H=16 q-heads, KV=4 kv-heads, DK=64.
Returns (output [B,S,D], attention_weights [B,H,S,S]) like the reference.

Sharding: tensor-parallel over q-heads -- 2 q-heads per core, each core
pair shares one kv head (replicated projection).  w_q/w_k/w_v column
parallel, w_o row-parallel with the all-reduce done as a host-side sum of
per-core partial outputs.

Device-side layout trick: everything is computed transposed.  The host
supplies x^T for q/k/v so projections need no on-device transpose:
  QT [128, 4096]  (q-dims of this core's 2 heads  x  b*S+s)
  KT [64, 4096]
  Vaug [t, 65]    (V projection + a ones column)
Scores are built as exp(scoresT[t, s]); the AV matmul contracts over the
partition (t) axis, and the appended ones column of Vaug yields the
softmax denominator for free.  Attention weights are written to DRAM
transposed ([t,s]) and un-transposed on the host.
"""

import os
import sys

import numpy as np

for _p in ("/opt/trn_rl_repo", "/root/.axon_site/_ro/trn_rl_repo"):
    if os.path.isdir(_p) and _p not in sys.path:
        sys.path.insert(0, _p)

import concourse.bass as bass
import concourse.tile as tile
from concourse import bacc, mybir
from concourse.bass_utils import run_bass_kernel_spmd
from concourse.masks import make_identity

B, S, D = 2, 2048, 1024
H, KV, DK = 16, 4, 64
HPC = 2            # q heads per core
QD = HPC * DK      # 128 q-projection dims per core
BS = B * S         # 4096
NCORES = 8
F32 = mybir.dt.float32

_nc_cache = None
last_exec_time_ns = None
last_profile = None


def _build_program():
    nc = bacc.Bacc()

    xqT = nc.declare_dram_parameter("xqT", [D, BS], F32, isOutput=False)
    xkT = nc.declare_dram_parameter("xkT", [D, BS], F32, isOutput=False)
    xvT = nc.declare_dram_parameter("xvT", [D, BS], F32, isOutput=False)
    wq = nc.declare_dram_parameter("wq", [D, QD], F32, isOutput=False)
    bq = nc.declare_dram_parameter("bq", [QD, 1], F32, isOutput=False)
    wk = nc.declare_dram_parameter("wk", [D, DK], F32, isOutput=False)
    bk = nc.declare_dram_parameter("bk", [QD, 1], F32, isOutput=False)
    wv = nc.declare_dram_parameter("wv", [D, DK], F32, isOutput=False)
    bv = nc.declare_dram_parameter("bv", [DK, 1], F32, isOutput=False)
    wo = nc.declare_dram_parameter("wo", [QD, D], F32, isOutput=False)
    bo8 = nc.declare_dram_parameter("bo8", [1, D], F32, isOutput=False)

    attn_t = nc.declare_dram_parameter("attn_t", [B, HPC, S, S], F32, isOutput=True)
    out_o = nc.declare_dram_parameter("out_o", [BS, D], F32, isOutput=True)

    KT = D // 128      # 8 contraction tiles for the projections
    NT = S // 128      # 16 t-tiles per batch
    TT = BS // 128     # 32 t-tiles overall
    VW = DK + 1        # 65: V plus ones column

    with tile.TileContext(nc) as tc:
        with tc.tile_pool(name="singles", bufs=1) as singles:
            # ---- weights to SBUF -------------------------------------------------
            wq_sb = singles.tile([128, D], F32)       # k-tile i at [:, i*QD:(i+1)*QD]
            wk_sb = singles.tile([128, KT * DK], F32)
            wv_sb = singles.tile([128, KT * DK], F32)
            wo_sb = singles.tile([128, D], F32)
            for i in range(KT):
                nc.sync.dma_start(out=wq_sb[:, i * QD:(i + 1) * QD],
                                  in_=wq[i * 128:(i + 1) * 128, :])
                nc.sync.dma_start(out=wk_sb[:, i * DK:(i + 1) * DK],
                                  in_=wk[i * 128:(i + 1) * 128, :])
                nc.sync.dma_start(out=wv_sb[:, i * DK:(i + 1) * DK],
                                  in_=wv[i * 128:(i + 1) * 128, :])
            nc.sync.dma_start(out=wo_sb, in_=wo[:, :])
            bq_sb = singles.tile([QD, 1], F32)
            bk_sb = singles.tile([QD, 1], F32)
            bv_sb = singles.tile([DK, 1], F32)
            bo8_sb = singles.tile([1, D], F32)
            nc.sync.dma_start(out=bq_sb, in_=bq[:, :])
            nc.sync.dma_start(out=bk_sb, in_=bk[:, :])
            nc.sync.dma_start(out=bv_sb, in_=bv[:, :])
            nc.sync.dma_start(out=bo8_sb, in_=bo8[:, :])
            ones_sb = singles.tile([1, 128], F32)
            nc.vector.memset(ones_sb, 1.0)
            ident_sb = singles.tile([DK, DK], F32)
            make_identity(nc, ident_sb)

            # ---- projections -----------------------------------------------------
            qt_sb = singles.tile([QD, BS], F32)       # QT (scaled by 1/8 on host)
            kt_sb = singles.tile([QD, BS], F32)       # KT replicated in both halves
            vaug_sb = singles.tile([128, TT * VW], F32)  # V in [t,d] + ones col
            avt_sb = singles.tile([128, BS], F32)     # normalized (attn@V)^T

            nc.vector.memset(vaug_sb, 1.0)

            # Q projection: qt = wq^T @ xqT  (accumulate over 8 k-tiles in PSUM)
            with tc.tile_pool(name="xq", bufs=3) as xpool, \
                 tc.tile_pool(name="psq", bufs=8, space="PSUM") as pspool:
                psq = [pspool.tile([QD, 512], F32, name="psq", tag="psq") for _ in range(8)]
                for i in range(KT):
                    xt = xpool.tile([128, BS], F32, tag="x")
                    nc.sync.dma_start(out=xt, in_=xqT[i * 128:(i + 1) * 128, :])
                    for j in range(8):
                        nc.tensor.matmul(psq[j],
                                         lhsT=wq_sb[:, i * QD:(i + 1) * QD],
                                         rhs=xt[:, j * 512:(j + 1) * 512],
                                         start=(i == 0), stop=(i == KT - 1))
                for j in range(8):
                    nc.scalar.activation(out=qt_sb[:, j * 512:(j + 1) * 512],
                                         in_=psq[j],
                                         func=mybir.ActivationFunctionType.Identity,
                                         bias=bq_sb, scale=1.0)

            # K projection
            with tc.tile_pool(name="xk", bufs=3) as xpool, \
                 tc.tile_pool(name="psk", bufs=8, space="PSUM") as pspool:
                psk = [pspool.tile([QD, 512], F32, name="psk", tag="psk") for _ in range(8)]
                for i in range(KT):
                    xt = xpool.tile([128, BS], F32, tag="x")
                    nc.sync.dma_start(out=xt, in_=xkT[i * 128:(i + 1) * 128, :])
                    for j in range(8):
                        # replicate K into both partition halves so head 1's
                        # scores matmul (rhs at base partition 64) has a
                        # base-matched lhsT
                        for h in range(2):
                            nc.tensor.matmul(psk[j][h * DK:(h + 1) * DK, :],
                                             lhsT=wk_sb[:, i * DK:(i + 1) * DK],
                                             rhs=xt[:, j * 512:(j + 1) * 512],
                                             start=(i == 0), stop=(i == KT - 1))
                for j in range(8):
                    nc.scalar.activation(out=kt_sb[:, j * 512:(j + 1) * 512],
                                         in_=psk[j],
                                         func=mybir.ActivationFunctionType.Identity,
                                         bias=bk_sb, scale=1.0)

            # V projection in [d, t] orientation (like K), then PE-transpose
            # into the [t, d] tiles of vaug_sb.
            with tc.tile_pool(name="vt", bufs=1) as vtpool:
                vt_tmp = vtpool.tile([DK, BS], F32)
                with tc.tile_pool(name="xv", bufs=3) as xpool, \
                     tc.tile_pool(name="psv", bufs=8, space="PSUM") as pspool:
                    psv = [pspool.tile([DK, 512], F32, name="psv", tag="psv")
                           for _ in range(8)]
                    for i in range(KT):
                        xt = xpool.tile([128, BS], F32, tag="x")
                        nc.sync.dma_start(out=xt, in_=xvT[i * 128:(i + 1) * 128, :])
                        for j in range(8):
                            nc.tensor.matmul(psv[j],
                                             lhsT=wv_sb[:, i * DK:(i + 1) * DK],
                                             rhs=xt[:, j * 512:(j + 1) * 512],
                                             start=(i == 0), stop=(i == KT - 1))
                    for j in range(8):
                        nc.scalar.activation(out=vt_tmp[:, j * 512:(j + 1) * 512],
                                             in_=psv[j],
                                             func=mybir.ActivationFunctionType.Identity,
                                             bias=bv_sb, scale=1.0)
                with tc.tile_pool(name="pst", bufs=4, space="PSUM") as pstpool:
                    for t in range(TT):
                        pst = pstpool.tile([128, DK], F32, tag="pst")
                        nc.tensor.transpose(pst, vt_tmp[:, t * 128:(t + 1) * 128],
                                            ident_sb)
                        nc.scalar.copy(out=vaug_sb[:, t * VW:t * VW + DK], in_=pst)

            # ---- attention -------------------------------------------------------
            SH = 1024   # s-half width
            with tc.tile_pool(name="expt", bufs=20) as expt_pool, \
                 tc.tile_pool(name="bc", bufs=2) as bc_pool, \
                 tc.tile_pool(name="inv", bufs=2) as inv_pool, \
                 tc.tile_pool(name="pss", bufs=4, space="PSUM") as pss_pool, \
                 tc.tile_pool(name="psav", bufs=2, space="PSUM") as psav_pool:
                for b in range(B):
                    for hl in range(HPC):
                        for half in range(2):
                            s0 = b * S + half * SH     # column into BS axis
                            av = psav_pool.tile([VW, SH], F32, tag="av")
                            ets = []
                            for t in range(NT):
                                tg = b * S + t * 128   # global t offset
                                et = expt_pool.tile([128, SH], F32, tag="et")
                                for ss in range(2):
                                    ps = pss_pool.tile([128, 512], F32, tag="ps")
                                    nc.tensor.matmul(
                                        ps,
                                        lhsT=kt_sb[hl * DK:(hl + 1) * DK,
                                                   tg:tg + 128],
                                        rhs=qt_sb[hl * DK:(hl + 1) * DK,
                                                  s0 + ss * 512:s0 + (ss + 1) * 512],
                                        start=True, stop=True)
                                    nc.scalar.activation(
                                        out=et[:, ss * 512:(ss + 1) * 512], in_=ps,
                                        func=mybir.ActivationFunctionType.Exp)
                                vt = (b * NT + t) * VW
                                for ss in range(2):
                                    nc.tensor.matmul(
                                        av[:, ss * 512:(ss + 1) * 512],
                                        lhsT=vaug_sb[:, vt:vt + VW],
                                        rhs=et[:, ss * 512:(ss + 1) * 512],
                                        start=(t == 0), stop=(t == NT - 1))
                                ets.append(et)
                                del et
                            # denominators -> reciprocal -> broadcast to 128 parts
                            inv = inv_pool.tile([1, SH], F32, tag="inv")
                            nc.vector.reciprocal(inv, av[DK:VW, :])
                            bc = bc_pool.tile([128, SH], F32, tag="bc")
                            for ss in range(2):
                                psb = pss_pool.tile([128, 512], F32, tag="ps")
                                nc.tensor.matmul(psb, lhsT=ones_sb,
                                                 rhs=inv[:, ss * 512:(ss + 1) * 512],
                                                 start=True, stop=True)
                                nc.scalar.copy(out=bc[:, ss * 512:(ss + 1) * 512],
                                               in_=psb)
                            # normalized AV^T slice into resident avt_sb
                            nc.vector.tensor_mul(
                                avt_sb[hl * DK:(hl + 1) * DK, s0:s0 + SH],
                                av[0:DK, :], bc[0:DK, :])
                            # normalize attention rows + store (transposed planes)
                            for t in range(NT):
                                nc.vector.tensor_mul(ets[t], ets[t], bc)
                                nc.sync.dma_start(
                                    out=attn_t[b, hl, t * 128:(t + 1) * 128,
                                               half * SH:(half + 1) * SH],
                                    in_=ets[t])

            # ---- output projection ----------------------------------------------
            with tc.tile_pool(name="oo", bufs=4) as oo_pool, \
                 tc.tile_pool(name="pso", bufs=4, space="PSUM") as pso_pool:
                bo_bc = singles.tile([128, D], F32)
                for e in range(2):
                    psb = pso_pool.tile([128, 512], F32, tag="pso")
                    nc.tensor.matmul(psb, lhsT=ones_sb,
                                     rhs=bo8_sb[:, e * 512:(e + 1) * 512],
                                     start=True, stop=True)
                    nc.scalar.copy(out=bo_bc[:, e * 512:(e + 1) * 512], in_=psb)
                for st in range(TT):
                    oo = oo_pool.tile([128, D], F32, tag="oo")
                    for e in range(2):
                        ps = pso_pool.tile([128, 512], F32, tag="pso")
                        nc.tensor.matmul(ps,
                                         lhsT=avt_sb[:, st * 128:(st + 1) * 128],
                                         rhs=wo_sb[:, e * 512:(e + 1) * 512],
                                         start=True, stop=True)
                        nc.vector.tensor_add(oo[:, e * 512:(e + 1) * 512], ps,
                                             bo_bc[:, e * 512:(e + 1) * 512])
                    nc.sync.dma_start(out=out_o[st * 128:(st + 1) * 128, :], in_=oo)

    nc.compile()
    return nc


def _get_program():
    global _nc_cache
    if _nc_cache is None:
        _nc_cache = _build_program()
    return _nc_cache


def kernel(**inputs):
    global last_exec_time_ns, last_profile
    q = np.ascontiguousarray(np.asarray(inputs["query"], np.float32).reshape(BS, D).T)
    k = np.ascontiguousarray(np.asarray(inputs["key"], np.float32).reshape(BS, D).T)
    v = np.ascontiguousarray(np.asarray(inputs["value"], np.float32).reshape(BS, D).T)
    w_q = np.asarray(inputs["w_q"], np.float32)
    b_q = np.asarray(inputs["b_q"], np.float32)
    w_k = np.asarray(inputs["w_k"], np.float32)
    b_k = np.asarray(inputs["b_k"], np.float32)
    w_v = np.asarray(inputs["w_v"], np.float32)
    b_v = np.asarray(inputs["b_v"], np.float32)
    w_o = np.asarray(inputs["w_o"], np.float32)
    b_o = np.asarray(inputs["b_o"], np.float32)

    scale = np.float32(1.0 / np.sqrt(DK))
    bo8 = (b_o / np.float32(NCORES)).reshape(1, D).astype(np.float32)

    in_maps = []
    for c in range(NCORES):
        kv = c // 2
        qs = slice(c * QD, (c + 1) * QD)
        ks = slice(kv * DK, (kv + 1) * DK)
        in_maps.append({
            "xqT": q, "xkT": k, "xvT": v,
            "wq": np.ascontiguousarray(w_q[:, qs]) * scale,
            "bq": (b_q[qs] * scale).reshape(QD, 1).astype(np.float32),
            "wk": np.ascontiguousarray(w_k[:, ks]),
            "bk": np.tile(b_k[ks], 2).reshape(QD, 1).astype(np.float32),
            "wv": np.ascontiguousarray(w_v[:, ks]),
            "bv": b_v[ks].reshape(DK, 1).astype(np.float32),
            "wo": np.ascontiguousarray(w_o[qs, :]),
            "bo8": bo8,
        })

    nc = _get_program()
    trace = bool(int(os.environ.get("KERNEL_TRACE", "0")))
    res = run_bass_kernel_spmd(nc, in_maps, list(range(NCORES)), trace=trace)
    last_exec_time_ns = res.exec_time_ns
    last_profile = res

    attn = np.empty((B, H, S, S), np.float32)
    out = np.zeros((BS, D), np.float32)
    for c in range(NCORES):
        r = res.results[c]
        attn[:, c * HPC:(c + 1) * HPC] = r["attn_t"].transpose(0, 1, 3, 2)
        out += r["out_o"]
    return out.reshape(B, S, D), attn
